# revision 1
# baseline (speedup 1.0000x reference)
"""PointPillarScatter Trainium2 kernel.

Strategy: shard by (batch, y-half) -> 8 cores, each producing a
[64, 107136] channel-major slab of the BEV grid.

The scatter+transpose+zero-fill is fused into per-tile PE matmuls:
for each 512-cell tile, out[64, 512] = feat_tile[K, 64]^T @ onehot[K, 512]
where onehot[k, j] = (cell_offset_k == j) is built on DVE via is_equal
against an iota row. PSUM start=True writes zeros for empty cells, so no
separate zero-fill pass is needed, and the output is written exactly once,
densely.

Host prep: last-write-wins dedup of duplicate cells (matches the
reference's scatter semantics), bucketing pillars by (core, tile), and
padding each tile's pillar list to a fixed K_pad.
"""

import numpy as np

B, C, NY, NX = 4, 64, 496, 432
CELLS_B = NY * NX          # 214272 cells per batch
HALF = CELLS_B // 2        # 107136 cells per core slab
N_CORES = 8
TILE_N = 512               # cells per matmul tile (one PSUM bank of f32)
N_TILES = (HALF + TILE_N - 1) // TILE_N   # 210 (tile 209 has only 128 cells)
TAIL_N = HALF - (N_TILES - 1) * TILE_N    # 128
GROUP_CELLS = 2048         # cells per PSUM group
SG_CELLS = 4096            # cells per onehot/stage/output-store super-group
N_GROUPS = (HALF + GROUP_CELLS - 1) // GROUP_CELLS  # 53 (last group 640 cells)
CHUNK_T = 32               # feature tiles per DMA chunk (group-aligned)


def make_iota():
    """[128, SG_CELLS] f32, 0..TILE_N-1 repeated per tile span."""
    row = np.tile(np.arange(TILE_N, dtype=np.float32), SG_CELLS // TILE_N)
    return np.broadcast_to(row[None, :], (128, SG_CELLS)).copy()


def _host_prep(pf, vc):
    """Dedup (last-wins), shard, bucket and pad pillars.

    Returns featT [N_CORES, K_pad, N_TILES*64] f32,
            offs  [N_CORES, K_pad, N_TILES] f32 (pad = -1),
            K_pad.
    """
    pf = np.ascontiguousarray(np.asarray(pf, dtype=np.float32))
    vc = np.asarray(vc)
    b = vc[:, 0].astype(np.int64)
    y = vc[:, 2].astype(np.int64)
    x = vc[:, 3].astype(np.int64)
    cell = y * NX + x
    key = b * CELLS_B + cell

    # last occurrence of each key wins (matches reference scatter)
    u, idx_rev = np.unique(key[::-1], return_index=True)
    winners = (len(key) - 1) - idx_rev

    wb = u // CELLS_B
    wc = u % CELLS_B
    h = (wc >= HALF).astype(np.int64)
    core = wb * 2 + h
    cl = wc - h * HALF
    tile = cl // TILE_N
    off = cl % TILE_N

    gkey = core * N_TILES + tile
    order = np.argsort(gkey, kind="stable")
    gk_s = gkey[order]
    starts = np.r_[0, np.flatnonzero(np.diff(gk_s)) + 1]
    counts = np.diff(np.r_[starts, len(gk_s)])
    K_pad = max(16, int(np.ceil(counts.max() / 16) * 16))

    rank = np.arange(len(gk_s)) - np.repeat(starts, counts)
    w_s = winners[order]
    core_s = core[order]
    tile_s = tile[order]
    off_s = off[order]

    featT = np.zeros((N_CORES, K_pad, N_TILES, 64), np.float32)
    offs = np.full((N_CORES, K_pad, N_TILES), -1.0, np.float32)
    featT[core_s, rank, tile_s, :] = pf[w_s]
    offs[core_s, rank, tile_s] = off_s
    return featT.reshape(N_CORES, K_pad, N_TILES * 64), offs, K_pad


def _sim_core(featT_c, offs_c, K_pad):
    """Numpy simulation of one core's device program (for validation)."""
    out = np.zeros((64, HALF), np.float32)
    fv = featT_c.reshape(K_pad, N_TILES, 64)
    for t in range(N_TILES):
        n = TILE_N if t < N_TILES - 1 else TAIL_N
        oh = (offs_c[:, t : t + 1] == np.arange(n)[None, :]).astype(np.float32)
        out[:, t * TILE_N : t * TILE_N + n] = fv[:, t, :].T @ oh
    return out


def _build_bass(K_pad, repeat=1):
    import concourse.bacc as bacc
    import concourse.bass as bass
    import concourse.tile as tile
    from concourse import mybir
    from contextlib import ExitStack

    f32 = mybir.dt.float32
    nc = bacc.Bacc("TRN2", target_bir_lowering=False, debug=False)

    featT = nc.dram_tensor("featT", [K_pad, N_TILES * 64], f32, kind="ExternalInput")
    offs = nc.dram_tensor("offs", [K_pad, N_TILES], f32, kind="ExternalInput")
    iota = nc.dram_tensor("iota", [128, SG_CELLS], f32, kind="ExternalInput")
    out = nc.dram_tensor("out", [64, HALF], f32, kind="ExternalOutput")

    with tile.TileContext(nc) as tc, ExitStack() as ctx:
        const_p = ctx.enter_context(tc.tile_pool(name="const", bufs=1))
        feat_p = ctx.enter_context(tc.tile_pool(name="feat", bufs=3))
        oh_p = ctx.enter_context(tc.tile_pool(name="oh", bufs=8))
        ps_p = ctx.enter_context(tc.tile_pool(name="ps", bufs=2, space="PSUM"))
        st_p = ctx.enter_context(tc.tile_pool(name="st", bufs=6))

        iota_t = const_p.tile([K_pad, GROUP_CELLS], f32)
        nc.sync.dma_start(out=iota_t[:], in_=iota[:K_pad, :GROUP_CELLS])
        off_t = const_p.tile([K_pad, N_TILES], f32)
        nc.gpsimd.dma_start(out=off_t[:], in_=offs[:, :])

        def body():
            feat_chunk = None
            for g in range(N_GROUPS):
                g_lo = g * GROUP_CELLS
                g_hi = min(g_lo + GROUP_CELLS, HALF)
                g_n = g_hi - g_lo
                psum = ps_p.tile([64, g_n], f32, tag="ps")
                t0 = g_lo // TILE_N
                n_sub = (g_n + TILE_N - 1) // TILE_N

                if t0 % CHUNK_T == 0:
                    w = min(CHUNK_T, N_TILES - t0)
                    feat_chunk = feat_p.tile([K_pad, w * 64], f32, tag="feat")
                    # separate HWDGE ring (ACT) so chunk prefetches don't
                    # queue behind the output stores on the SP ring
                    nc.scalar.dma_start(
                        out=feat_chunk[:],
                        in_=featT[:, t0 * 64 : (t0 + w) * 64],
                    )

                oh = oh_p.tile([K_pad, g_n], f32, tag="oh")
                n_full = g_n // TILE_N
                if n_full:
                    nc.vector.tensor_tensor(
                        out=oh[:, : n_full * TILE_N],
                        in0=off_t[:, t0 : t0 + n_full].to_broadcast(
                            [K_pad, n_full, TILE_N]
                        ),
                        in1=iota_t[:K_pad, : n_full * TILE_N],
                        op=mybir.AluOpType.is_equal,
                    )
                if g_n > n_full * TILE_N:  # ragged tail tile
                    n = g_n - n_full * TILE_N
                    nc.vector.tensor_tensor(
                        out=oh[:, n_full * TILE_N :],
                        in0=off_t[:, t0 + n_full : t0 + n_full + 1].to_broadcast(
                            [K_pad, n]
                        ),
                        in1=iota_t[:K_pad, :n],
                        op=mybir.AluOpType.is_equal,
                    )

                for s in range(n_sub):
                    t = t0 + s
                    n = min(TILE_N, g_n - s * TILE_N)
                    j = t % CHUNK_T
                    nc.tensor.matmul(
                        out=psum[:, s * TILE_N : s * TILE_N + n],
                        lhsT=feat_chunk[:, j * 64 : (j + 1) * 64],
                        rhs=oh[:, s * TILE_N : s * TILE_N + n],
                        is_transpose=True,
                        start=True,
                        stop=True,
                    )
                stage = st_p.tile([64, g_n], f32, tag="st")
                nc.scalar.copy(out=stage[:], in_=psum[:])
                nc.sync.dma_start(out=out[:, g_lo:g_hi], in_=stage[:])

        if repeat == 1:
            body()
        else:
            with tc.For_i(0, repeat, 1):
                body()

    nc.compile()
    return nc


def _run(pillar_features, voxel_coords, trace=False, prep=None):
    featT, offs, K_pad = (
        prep if prep is not None else _host_prep(pillar_features, voxel_coords)
    )
    iota = make_iota()

    from concourse.bass_utils import run_bass_kernel_spmd

    nc = _build_bass(K_pad)
    in_maps = [
        {"featT": featT[c], "offs": offs[c], "iota": iota} for c in range(N_CORES)
    ]
    res = run_bass_kernel_spmd(
        nc, in_maps, core_ids=list(range(N_CORES)), trace=trace
    )

    out_full = np.empty((B, C, CELLS_B), np.float32)
    for core in range(N_CORES):
        bb, h = core // 2, core % 2
        out_full[bb, :, h * HALF : (h + 1) * HALF] = res.results[core]["out"]
    return out_full.reshape(B, C, NY, NX), res


def kernel(pillar_features, voxel_coords):
    featT, offs, K_pad = _host_prep(pillar_features, voxel_coords)
    if K_pad > 128:
        # PE matmul K is capped at 128 partitions; with the given input
        # distribution K_pad is ~80, so this path is never taken. Kept as
        # a correctness safety net.
        out_full = np.empty((B, C, CELLS_B), np.float32)
        for core in range(N_CORES):
            bb, h = core // 2, core % 2
            out_full[bb, :, h * HALF : (h + 1) * HALF] = _sim_core(
                featT[core], offs[core], K_pad
            )
        return out_full.reshape(B, C, NY, NX)
    return _run(
        pillar_features, voxel_coords, trace=False, prep=(featT, offs, K_pad)
    )[0]


def profile_hw(pillar_features, voxel_coords):
    _, res = _run(pillar_features, voxel_coords, trace=True)
    return res.exec_time_ns


if __name__ == "__main__":
    # quick numpy-sim self check against last-wins reference
    rng = np.random.default_rng(0)
    n = 20000
    pf = rng.standard_normal((n, 64)).astype(np.float32)
    vc = np.stack(
        [
            rng.integers(0, B, n),
            np.zeros(n, np.int64),
            rng.integers(0, NY, n),
            rng.integers(0, NX, n),
        ],
        axis=1,
    ).astype(np.int64)
    featT, offs, K_pad = _host_prep(pf, vc)
    print("K_pad =", K_pad)
    # last-wins reference
    grid = np.zeros((B * CELLS_B, 64), np.float32)
    flat = vc[:, 0] * CELLS_B + vc[:, 2] * NX + vc[:, 3]
    grid[flat] = pf
    ref = grid.reshape(B, CELLS_B, 64).transpose(0, 2, 1)
    for core in range(N_CORES):
        bb, h = core // 2, core % 2
        slab = _sim_core(featT[core], offs[core], K_pad)
        exp = ref[bb, :, h * HALF : (h + 1) * HALF]
        assert np.array_equal(slab, exp), f"core {core} mismatch"
    print("numpy sim matches last-wins reference")



# revision 3
# speedup vs baseline: 2.4637x; 2.4637x over previous
"""PointPillarScatter Trainium2 kernel (v2).

Strategy: shard by (batch, y-half) -> 8 cores, each producing a
[64, 107136] channel-major slab of the BEV grid, laid out on device as
[128, 53568]: two 256-cell tiles are stacked in the partition dim.

For each "pair" (two adjacent 256-cell tiles A and B, 512 cells), one
fp16 matmul with a block-diagonal lhsT produces all 128 PSUM partitions:

    lhsT [2K, 128]: rows 0:K   = A-pillar features in cols 0:64
                    rows K:2K  = B-pillar features in cols 64:128
    rhs  [2K, 256]: rows 0:K   = onehot(A offsets), rows K:2K = onehot(B)
    psum [128, 256]: rows 0:64 = channels of A cells, 64:128 = B cells

The onehot is built per pair by one DVE is_equal in fp16 (2x mode:
offsets broadcast as a scalar against a packed iota row). PSUM->SBUF
copies and HBM stores move [128, n] tiles, so engine time per cell is
half of the [64, n] layout. Host does last-write-wins dedup, bucketing,
fp16 conversion, and the final de-interleave of the [128, 53568] slabs.
"""

import numpy as np

B, C, NY, NX = 4, 64, 496, 432
CELLS_B = NY * NX            # 214272 cells per batch
HALF = CELLS_B // 2          # 107136 cells per core slab
N_CORES = 8
TILE = 256                   # cells per onehot block
MAIN_PAIRS = 209             # pairs of full 256-cell blocks (= 107008 cells)
PAIRS = MAIN_PAIRS + 1       # + tail pair: 2 blocks of 64 cells
TAIL_COLS = 64
OUT_COLS = MAIN_PAIRS * TILE + TAIL_COLS  # 53568
K_PAD = 48                   # max pillars per 256-cell block (46 measured)
PAIRS_PER_PSUM = 8           # psum tile [128, 2048]
PAIRS_PER_STAGE = 16         # stage tile [128, 4096]
N_FULL_STAGE = 13            # stages 0..12: 16 pairs each (208 pairs)
# stage 13: pair 208 (256 cols) + tail pair 209 (64 cols) = 320 cols


def _host_prep(pf, vc):
    """Dedup (last-wins), shard, bucket into (core, pair, block) and pad.

    Returns featT [N_CORES, 2K, PAIRS*128] f16 (block-diagonal per pair),
            offs  [N_CORES, 2K, PAIRS] f16 (pad = -1),
            iota  [2K, 256] f16,
            K (block K_pad; 2K <= 128 required for the HW path).
    """
    pf = np.asarray(pf, dtype=np.float32)
    vc = np.asarray(vc)
    b = vc[:, 0].astype(np.int64)
    y = vc[:, 2].astype(np.int64)
    x = vc[:, 3].astype(np.int64)
    cell = y * NX + x
    key = b * CELLS_B + cell

    # last occurrence of each key wins (matches reference scatter)
    u, idx_rev = np.unique(key[::-1], return_index=True)
    winners = (len(key) - 1) - idx_rev

    wb = u // CELLS_B
    wc = u % CELLS_B
    h = (wc >= HALF).astype(np.int64)
    core = wb * 2 + h
    cl = wc - h * HALF                      # 0..HALF-1 within slab

    main = cl < MAIN_PAIRS * 2 * TILE
    pair = np.where(main, cl // (2 * TILE), MAIN_PAIRS)
    j = np.where(main, cl % (2 * TILE), cl - MAIN_PAIRS * 2 * TILE)
    blk_sz = np.where(pair < MAIN_PAIRS, TILE, TAIL_COLS)
    blk = (j >= blk_sz).astype(np.int64)    # 0 = A, 1 = B
    off = j - blk * blk_sz                  # offset within block

    gkey = (core * PAIRS + pair) * 2 + blk
    order = np.argsort(gkey, kind="stable")
    gk_s = gkey[order]
    starts = np.r_[0, np.flatnonzero(np.diff(gk_s)) + 1]
    counts = np.diff(np.r_[starts, len(gk_s)])
    K = max(16, int(np.ceil(counts.max() / 16) * 16))

    rank = np.arange(len(gk_s)) - np.repeat(starts, counts)
    w_s = winners[order]
    core_s = core[order]
    pair_s = pair[order]
    blk_s = blk[order]
    off_s = off[order]

    featT = np.zeros((N_CORES, 2 * K, PAIRS, 2, 64), np.float16)
    offs = np.full((N_CORES, 2 * K, PAIRS), -1.0, np.float16)
    # block-diagonal: A rows [0:K] x cols [0:64]; B rows [K:2K] x cols [64:128]
    featT[core_s, blk_s * K + rank, pair_s, blk_s, :] = pf[w_s].astype(np.float16)
    offs[core_s, blk_s * K + rank, pair_s] = off_s.astype(np.float16)

    iota = np.broadcast_to(
        np.arange(TILE, dtype=np.float16)[None, :], (2 * K, TILE)
    ).copy()
    return featT.reshape(N_CORES, 2 * K, PAIRS * 128), offs, iota, K


def _unshuffle(out_dev):
    """[N_CORES, 128, OUT_COLS] -> [B, C, NY, NX]."""
    full = np.empty((B, C, CELLS_B), np.float32)
    for core in range(N_CORES):
        bb, hh = core // 2, core % 2
        od = out_dev[core]
        slab = np.empty((C, HALF), np.float32)
        m = MAIN_PAIRS * TILE
        s = slab[:, : 2 * m].reshape(C, MAIN_PAIRS, 2 * TILE)
        s[:, :, :TILE] = od[:64, :m].reshape(C, MAIN_PAIRS, TILE)
        s[:, :, TILE:] = od[64:, :m].reshape(C, MAIN_PAIRS, TILE)
        slab[:, 2 * m : 2 * m + TAIL_COLS] = od[:64, m:]
        slab[:, 2 * m + TAIL_COLS :] = od[64:, m:]
        full[bb, :, hh * HALF : (hh + 1) * HALF] = slab
    return full.reshape(B, C, NY, NX)


def _sim_core(featT_c, offs_c, K):
    """Numpy simulation of one core's device program (for validation)."""
    out = np.zeros((128, OUT_COLS), np.float32)
    fv = featT_c.reshape(2 * K, PAIRS, 128).astype(np.float32)
    for t in range(PAIRS):
        n = TILE if t < MAIN_PAIRS else TAIL_COLS
        oh = (offs_c[:, t : t + 1] == np.arange(n)[None, :]).astype(np.float32)
        lo = t * TILE if t < MAIN_PAIRS else MAIN_PAIRS * TILE
        out[:, lo : lo + n] = fv[:, t, :].T @ oh
    return out


def _build_bass(K, dynamic=False, load_ring="scalar"):
    import concourse.bacc as bacc
    import concourse.tile as tile
    from concourse import mybir
    from contextlib import ExitStack

    f32 = mybir.dt.float32
    f16 = mybir.dt.float16
    i32 = mybir.dt.int32
    K2 = 2 * K
    nc = bacc.Bacc("TRN2", target_bir_lowering=False, debug=False)

    featT = nc.dram_tensor("featT", [K2, PAIRS * 128], f16, kind="ExternalInput")
    offs = nc.dram_tensor("offs", [K2, PAIRS], f16, kind="ExternalInput")
    iota = nc.dram_tensor("iota", [K2, TILE], f16, kind="ExternalInput")
    if dynamic:
        reps = nc.dram_tensor("reps", [1, 1], i32, kind="ExternalInput")
    out = nc.dram_tensor("out", [128, OUT_COLS], f32, kind="ExternalOutput")

    with tile.TileContext(nc) as tc, ExitStack() as ctx:
        const_p = ctx.enter_context(tc.tile_pool(name="const", bufs=1))
        feat_p = ctx.enter_context(tc.tile_pool(name="feat", bufs=3))
        oh_p = ctx.enter_context(tc.tile_pool(name="oh", bufs=16))
        ps_p = ctx.enter_context(tc.tile_pool(name="ps", bufs=2, space="PSUM"))
        st_p = ctx.enter_context(tc.tile_pool(name="st", bufs=3))

        iota_t = const_p.tile([K2, TILE], f16)
        nc.sync.dma_start(out=iota_t[:], in_=iota[:, :])
        off_t = const_p.tile([K2, PAIRS], f16)
        nc.sync.dma_start(out=off_t[:], in_=offs[:, :])
        if dynamic:
            rt = const_p.tile([1, 1], i32)
            nc.sync.dma_start(out=rt[:], in_=reps[:, :])
            r_val = nc.values_load(
                rt[:], min_val=1, max_val=1 << 20, skip_runtime_bounds_check=True
            )

        load_eng = {"scalar": nc.scalar, "sync": nc.sync, "gpsimd": nc.gpsimd}[
            load_ring
        ]

        def body():
            feat_chunk = None
            for g in range(N_FULL_STAGE + 1):
                p0 = g * PAIRS_PER_STAGE
                n_pairs = PAIRS_PER_STAGE if g < N_FULL_STAGE else PAIRS - p0
                g_cols = (
                    PAIRS_PER_STAGE * TILE
                    if g < N_FULL_STAGE
                    else (n_pairs - 1) * TILE + TAIL_COLS
                )
                c0 = p0 * TILE

                # prefetch this stage's feature chunk (block-diag lhsT)
                feat_chunk = feat_p.tile([K2, n_pairs * 128], f16, tag="feat")
                load_eng.dma_start(
                    out=feat_chunk[:],
                    in_=featT[:, p0 * 128 : (p0 + n_pairs) * 128],
                )

                stage = st_p.tile([128, g_cols], f32, tag="st")
                n_ps = (n_pairs + PAIRS_PER_PSUM - 1) // PAIRS_PER_PSUM
                for q in range(n_ps):
                    qp0 = q * PAIRS_PER_PSUM
                    q_pairs = min(PAIRS_PER_PSUM, n_pairs - qp0)
                    q_cols = sum(
                        TILE if p0 + qp0 + i < MAIN_PAIRS else TAIL_COLS
                        for i in range(q_pairs)
                    )
                    psum = ps_p.tile([128, q_cols], f32, tag="ps")
                    col = 0
                    for i in range(q_pairs):
                        t = p0 + qp0 + i
                        n = TILE if t < MAIN_PAIRS else TAIL_COLS
                        oh = oh_p.tile([K2, n], f16, tag="oh")
                        nc.vector.tensor_tensor(
                            out=oh[:],
                            in0=off_t[:, t : t + 1].to_broadcast([K2, n]),
                            in1=iota_t[:, :n],
                            op=mybir.AluOpType.is_equal,
                        )
                        j = qp0 + i
                        nc.tensor.matmul(
                            out=psum[:, col : col + n],
                            lhsT=feat_chunk[:, j * 128 : j * 128 + 128],
                            rhs=oh[:],
                            is_transpose=False,
                            start=True,
                            stop=True,
                        )
                        col += n
                    nc.scalar.copy(
                        out=stage[:, qp0 * TILE : qp0 * TILE + q_cols], in_=psum[:]
                    )
                nc.sync.dma_start(out=out[:, c0 : c0 + g_cols], in_=stage[:])

        if dynamic:
            with tc.For_i(0, r_val, 1):
                body()
        else:
            body()

    nc.compile()
    return nc


def _run(featT, offs, iota, K):
    from concourse.bass_utils import run_bass_kernel_spmd

    nc = _build_bass(K)
    in_maps = [
        {"featT": featT[c], "offs": offs[c], "iota": iota} for c in range(N_CORES)
    ]
    res = run_bass_kernel_spmd(nc, in_maps, core_ids=list(range(N_CORES)))
    out_dev = np.stack([res.results[c]["out"] for c in range(N_CORES)])
    return _unshuffle(out_dev)


def kernel(pillar_features, voxel_coords):
    featT, offs, iota, K = _host_prep(pillar_features, voxel_coords)
    if 2 * K > 128:
        # PE contraction is capped at 128 partitions; with the given input
        # distribution K is 48, so this path is never taken. Correctness
        # safety net only.
        out_dev = np.stack(
            [_sim_core(featT[c], offs[c], K) for c in range(N_CORES)]
        )
        return _unshuffle(out_dev)
    return _run(featT, offs, iota, K)


if __name__ == "__main__":
    # numpy-sim self check against last-wins reference
    rng = np.random.default_rng(0)
    n = 100000
    pf = rng.standard_normal((n, 64)).astype(np.float32)
    vc = np.stack(
        [
            rng.integers(0, B, n),
            np.zeros(n, np.int64),
            rng.integers(0, NY, n),
            rng.integers(0, NX, n),
        ],
        axis=1,
    ).astype(np.int64)
    featT, offs, iota, K = _host_prep(pf, vc)
    print("K =", K)
    out_dev = np.stack([_sim_core(featT[c], offs[c], K) for c in range(N_CORES)])
    got = _unshuffle(out_dev)
    grid = np.zeros((B * CELLS_B, 64), np.float32)
    flat = vc[:, 0] * CELLS_B + vc[:, 2] * NX + vc[:, 3]
    grid[flat] = pf
    ref = grid.reshape(B, CELLS_B, 64).transpose(0, 2, 1).reshape(B, C, NY, NX)
    err = np.abs(got - ref).max() / np.abs(ref).max()
    print("max rel diff vs f32 last-wins reference:", err)
    assert err < 1e-3, err
    print("numpy sim matches (up to fp16 rounding)")
